# revision 15
# baseline (speedup 1.0000x reference)
"""Weighted 2D cross-entropy (BCE-over-classes) loss on 8 Trainium2 cores.

Math (matches the reference):
  t in [0,19); pos = t>0, neg = t==0 (all pixels are pos or neg; mask == 1)
  S(i) = sum_c bce(i,c) = -[ A(i) + B(i) ]
     A(i)   = sum_c log(1-p_c(i))
     B(i)   = log(p_t(i)) - log(1-p_t(i)) = Ln(exp(-L_sel(i)) - 1)
  loss = ( (NEG/TOT)*S_pos_sum + (POS/TOT)*S_neg_sum ) / (TOT*C)

Per-core (core k <- batch element k, pure data parallel), pixel grid
[128, 4096]. Full-grid instructions amortize per-op engine overheads;
only class 18 runs on quarters (short drain).
  per class: 2MB DMA; ACT L_c = Ln(1-p_c) f32->bf16 (accum_out -> U);
  DVE eq_c = (T==c) bf16 (4x, hoisted one class ahead), masked = eq*L
  (2x), A += L (bf16 ping-pong, 2x); PE identity-matmuls accumulate
  L_sel = sum_c masked into PSUM f32 (all 8 banks).
  tail per quarter: pos*A on GpSimd (off the critical chain), ACT
  expn = exp(-L_sel) x4 then B = Ln(expn-1) x4 (one table swap each;
  accum_out -> sum B), pos*B on DVE.
Host counts pos pixels from the input and folds the [128, 40] stats.
"""

from contextlib import ExitStack

import numpy as np

import concourse.bass as bass
import concourse.mybir as mybir
import concourse.tile as tile
from concourse import bacc
from concourse.bass_utils import run_bass_kernel_spmd

# problem shape (hardcoded per harness contract)
N, C, H, W = 8, 19, 512, 1024
PIX = H * W          # 524288 pixels per core
P = 128              # partitions
FCOLS = PIX // P     # 4096 free columns when pixels laid out [128, 4096]
QT = FCOLS // 4      # 1024-wide quarters (class 18 + tail)
N_CORES = 8

DT = mybir.dt

# stats column layout (all f32)
COL_U = 0            # 22 cols: sum L_c per class (c18 split in quarters)
COL_POSA = 22        # 4 cols: sum pos*A per quarter
COL_POSB = 26        # 4 cols: sum pos*B per quarter
COL_SUMB = 30        # 4 cols: sum B per quarter
NCOLS = 40           # padded


def build_kernel() -> bass.Bass:
    # Bacc (not raw Bass): its compile() pipeline runs
    # generate_event_semaphores, which splits multi-sem waits to satisfy the
    # 1-wait-per-instruction TRN2 sync structs.
    nc = bacc.Bacc("TRN2")

    predict = nc.declare_dram_parameter("predict", [C, PIX], DT.float32, isOutput=False)
    target = nc.declare_dram_parameter("target", [P, FCOLS], DT.int32, isOutput=False)
    idn = nc.declare_dram_parameter("idn", [P, P], DT.bfloat16, isOutput=False)
    out = nc.declare_dram_parameter("out", [P, NCOLS], DT.float32, isOutput=True)

    pred_r = predict.rearrange("c (p f) -> c p f", p=P)  # [19, 128, 4096]

    with tile.TileContext(nc) as tc, ExitStack() as ctx:
        const = ctx.enter_context(tc.tile_pool(name="const", bufs=1))
        p_pool = ctx.enter_context(tc.tile_pool(name="p", bufs=3))
        lm_pool = ctx.enter_context(tc.tile_pool(name="lm", bufs=4))
        eq_pool = ctx.enter_context(tc.tile_pool(name="eq", bufs=2))
        msk_pool = ctx.enter_context(tc.tile_pool(name="msk", bufs=3))
        a_pool = ctx.enter_context(tc.tile_pool(name="apool", bufs=2))
        tail_pool = ctx.enter_context(tc.tile_pool(name="tail", bufs=1))
        psum_pool = ctx.enter_context(tc.tile_pool(name="ps", bufs=1, space="PSUM"))

        idn_sb = const.tile([P, P], DT.bfloat16, tag="idn")
        nc.sync.dma_start(out=idn_sb[:], in_=idn[:])

        stats = const.tile([P, NCOLS], DT.float32, tag="stats")
        nc.vector.memset(stats[:], 0.0)

        # bias=-1.0 has no pre-registered const AP; build one
        neg1 = const.tile([P, 1], DT.float32, tag="neg1")
        nc.vector.memset(neg1[:], -1.0)

        t_i32 = const.tile([P, FCOLS], DT.int32, tag="ti")
        nc.sync.dma_start(out=t_i32[:], in_=target[:])
        t_bf = const.tile([P, FCOLS], DT.bfloat16, tag="tb")
        nc.vector.tensor_copy(out=t_bf[:], in_=t_i32[:])

        lsel_ps = psum_pool.tile([P, FCOLS], DT.float32, tag="lsel")

        a_prev = None
        eq_next = None

        def emit_eq(cls, cols_slice, tag="eq"):
            cols = cols_slice.stop - cols_slice.start
            eq = eq_pool.tile([P, cols], DT.bfloat16, tag=tag)
            nc.vector.tensor_scalar(
                out=eq[:],
                in0=t_bf[:, cols_slice],
                scalar1=float(cls),
                scalar2=None,
                op0=mybir.AluOpType.is_equal,
            )
            return eq

        # eq for class 0 up front
        eq_next = emit_eq(0, slice(0, FCOLS))

        # classes 0..17 full-grid
        for c in range(C - 1):
            p_t = p_pool.tile([P, FCOLS], DT.float32, tag="p")
            nc.sync.dma_start(out=p_t[:], in_=pred_r[c, :, :])
            lm = lm_pool.tile([P, FCOLS], DT.bfloat16, tag="lm")
            nc.scalar.activation(
                out=lm[:],
                in_=p_t[:],
                func=mybir.ActivationFunctionType.Ln,
                bias=1.0,
                scale=-1.0,
                accum_out=stats[:, COL_U + c : COL_U + c + 1],
            )
            eq = eq_next
            eq_next = emit_eq(c + 1, slice(0, FCOLS))
            msk = msk_pool.tile([P, FCOLS], DT.bfloat16, tag="msk")
            nc.vector.tensor_mul(out=msk[:], in0=eq[:], in1=lm[:])
            if c == 0:
                a_new = a_pool.tile([P, FCOLS], DT.bfloat16, tag="a")
                nc.vector.tensor_single_scalar(
                    out=a_new[:], in_=lm[:], scalar=1.0, op=mybir.AluOpType.mult
                )
            else:
                a_new = a_pool.tile([P, FCOLS], DT.bfloat16, tag="a")
                nc.vector.tensor_add(out=a_new[:], in0=a_prev[:], in1=lm[:])
            a_prev = a_new
            for s in range(FCOLS // 512):
                ssl = slice(s * 512, (s + 1) * 512)
                nc.tensor.matmul(
                    lsel_ps[:, ssl],
                    lhsT=idn_sb[:],
                    rhs=msk[:, ssl],
                    start=(c == 0),
                    stop=False,
                )

        # class 18 quartered; final A add per quarter; pos*A on GpSimd
        a_sb = const.tile([P, FCOLS], DT.bfloat16, tag="asbF")
        expn_q = []
        for q in range(4):
            qsl = slice(q * QT, (q + 1) * QT)
            p_t = p_pool.tile([P, QT], DT.float32, tag="p")
            nc.sync.dma_start(out=p_t[:], in_=pred_r[C - 1, :, qsl])
            lm = lm_pool.tile([P, QT], DT.bfloat16, tag="lm")
            nc.scalar.activation(
                out=lm[:],
                in_=p_t[:],
                func=mybir.ActivationFunctionType.Ln,
                bias=1.0,
                scale=-1.0,
                accum_out=stats[:, COL_U + 18 + q : COL_U + 19 + q],
            )
            eq18 = eq_next if q == 0 else emit_eq(C - 1, qsl)
            if q == 0:
                # eq_next was emitted full-grid for class 18; slice it
                eq18 = eq_next
                eqv = eq18[:, qsl]
            else:
                eqv = eq18[:]
            msk = msk_pool.tile([P, QT], DT.bfloat16, tag="msk")
            nc.vector.tensor_mul(out=msk[:], in0=eqv, in1=lm[:])
            nc.vector.tensor_add(out=a_sb[:, qsl], in0=a_prev[:, qsl], in1=lm[:])
            for s in range(2):
                ssl = slice(q * QT + s * 512, q * QT + (s + 1) * 512)
                nc.tensor.matmul(
                    lsel_ps[:, ssl],
                    lhsT=idn_sb[:],
                    rhs=msk[:, s * 512 : (s + 1) * 512],
                    start=False,
                    stop=True,
                )
            # pos*A (off the serial exp/ln drain chain)
            scr = tail_pool.tile([P, QT], DT.bfloat16, tag="scr")
            nc.vector.scalar_tensor_tensor(
                out=scr[:],
                in0=t_bf[:, qsl],
                scalar=0.5,
                in1=a_sb[:, qsl],
                op0=mybir.AluOpType.is_gt,
                op1=mybir.AluOpType.mult,
                accum_out=stats[:, COL_POSA + q : COL_POSA + q + 1],
            )
            # exp now; the B-Ln waits so all 4 exps share one table load
            expn = tail_pool.tile([P, QT], DT.float32, tag=f"expn{q}")
            nc.scalar.activation(
                out=expn[:],
                in_=lsel_ps[:, qsl],
                func=mybir.ActivationFunctionType.Exp,
                scale=-1.0,
            )
            expn_q.append(expn)

        # B = Ln(expn - 1) x4 (single Ln table load), then pos*B on DVE
        for q in range(4):
            qsl = slice(q * QT, (q + 1) * QT)
            b_t = tail_pool.tile([P, QT], DT.bfloat16, tag=f"b{q}")
            nc.scalar.activation(
                out=b_t[:],
                in_=expn_q[q][:],
                func=mybir.ActivationFunctionType.Ln,
                bias=neg1[:],
                accum_out=stats[:, COL_SUMB + q : COL_SUMB + q + 1],
            )
            scrb = tail_pool.tile([P, QT], DT.bfloat16, tag=f"scrb{q}")
            nc.vector.scalar_tensor_tensor(
                out=scrb[:],
                in0=t_bf[:, qsl],
                scalar=0.5,
                in1=b_t[:],
                op0=mybir.AluOpType.is_gt,
                op1=mybir.AluOpType.mult,
                accum_out=stats[:, COL_POSB + q : COL_POSB + q + 1],
            )

        nc.sync.dma_start(out=out[:], in_=stats[:])

    if not nc.is_finalized():
        nc.finalize()

    return nc


def combine(outs, pos_count, n_cores) -> np.float32:
    """Fold the cores' [128, 40] stats tiles into the scalar loss.
    pos_count is computed on host from the target input."""
    tot = np.float64(PIX) * n_cores
    pos = np.float64(pos_count)
    s_all = np.float64(0.0)
    s_pos = np.float64(0.0)
    for st in outs:
        st = st.astype(np.float64)
        u_all = st[:, COL_U : COL_U + 22].sum()
        pos_a = st[:, COL_POSA : COL_POSA + 4].sum()
        pos_b = st[:, COL_POSB : COL_POSB + 4].sum()
        sum_b = st[:, COL_SUMB : COL_SUMB + 4].sum()
        s_all += -(sum_b + u_all)
        s_pos += -(pos_b + pos_a)
    neg = tot - pos
    s_neg = s_all - s_pos
    loss = ((neg / tot) * s_pos + (pos / tot) * s_neg) / (tot * C)
    return np.float32(loss)


_NC_CACHE = None


def kernel(predict: np.ndarray, target: np.ndarray) -> np.ndarray:
    global _NC_CACHE
    if _NC_CACHE is None:
        _NC_CACHE = build_kernel()
    nc = _NC_CACHE

    import ml_dtypes

    predict = np.ascontiguousarray(predict, dtype=np.float32)
    target = np.ascontiguousarray(target, dtype=np.int32)
    idn = np.eye(P, dtype=np.float32).astype(ml_dtypes.bfloat16)

    in_maps = []
    for k in range(N_CORES):
        in_maps.append(
            {
                "predict": predict[k].reshape(C, PIX),
                "target": target[k].reshape(P, FCOLS),
                "idn": idn,
            }
        )

    res = run_bass_kernel_spmd(nc, in_maps, list(range(N_CORES)))
    pos_count = int(np.count_nonzero(target))
    return combine(
        [res.results[k]["out"] for k in range(N_CORES)], pos_count, N_CORES
    )


# revision 18
# speedup vs baseline: 1.0300x; 1.0300x over previous
"""Weighted 2D cross-entropy (BCE-over-classes) loss on 8 Trainium2 cores.

Math (matches the reference):
  t in [0,19); pos = t>0, neg = t==0 (all pixels are pos or neg; mask == 1)
  S(i) = sum_c bce(i,c) = -[ A(i) + B(i) ]
     A(i)   = sum_c log(1-p_c(i))
     B(i)   = log(p_t(i)) - log(1-p_t(i)) = Ln(exp(-L_sel(i)) - 1)
  loss = ( (NEG/TOT)*S_pos_sum + (POS/TOT)*S_neg_sum ) / (TOT*C)

Per-core (core k <- batch element k, pure data parallel), pixel grid
[128, 4096]. Full-grid instructions amortize per-op engine overheads;
only class 18 runs on quarters (short drain).
  per class: 2MB DMA; ACT L_c = Ln(1-p_c) f32->bf16 (accum_out -> U);
  DVE eq_c = (T==c) bf16 (4x, hoisted one class ahead), masked = eq*L
  (2x), A += L (bf16 ping-pong, 2x); PE identity-matmuls accumulate
  L_sel = sum_c masked into PSUM f32 (all 8 banks).
  tail per quarter: pos*A on GpSimd (off the critical chain), ACT
  expn = exp(-L_sel) x4 then B = Ln(expn-1) x4 (one table swap each;
  accum_out -> sum B), pos*B on DVE.
Host counts pos pixels from the input and folds the [128, 40] stats.
"""

from contextlib import ExitStack

import numpy as np

import concourse.bass as bass
import concourse.mybir as mybir
import concourse.tile as tile
from concourse import bacc
from concourse.bass_utils import run_bass_kernel_spmd

# problem shape (hardcoded per harness contract)
N, C, H, W = 8, 19, 512, 1024
PIX = H * W          # 524288 pixels per core
P = 128              # partitions
FCOLS = PIX // P     # 4096 free columns when pixels laid out [128, 4096]
QT = FCOLS // 4      # 1024-wide quarters (class 18 + tail)
N_CORES = 8

DT = mybir.dt

# stats column layout (all f32)
COL_U = 0            # 22 cols: sum L_c per class (c18 split in quarters)
COL_POSA = 22        # 4 cols: sum pos*A per quarter
COL_POSB = 26        # 4 cols: sum pos*B per quarter
COL_SUMB = 30        # 4 cols: sum B per quarter
NCOLS = 40           # padded


def build_kernel() -> bass.Bass:
    # Bacc (not raw Bass): its compile() pipeline runs
    # generate_event_semaphores, which splits multi-sem waits to satisfy the
    # 1-wait-per-instruction TRN2 sync structs.
    nc = bacc.Bacc("TRN2")

    predict = nc.declare_dram_parameter("predict", [C, PIX], DT.float32, isOutput=False)
    target = nc.declare_dram_parameter("target", [P, FCOLS], DT.int32, isOutput=False)
    idn = nc.declare_dram_parameter("idn", [P, P], DT.bfloat16, isOutput=False)
    out = nc.declare_dram_parameter("out", [P, NCOLS], DT.float32, isOutput=True)

    pred_r = predict.rearrange("c (p f) -> c p f", p=P)  # [19, 128, 4096]

    with tile.TileContext(nc) as tc, ExitStack() as ctx:
        const = ctx.enter_context(tc.tile_pool(name="const", bufs=1))
        p_pool = ctx.enter_context(tc.tile_pool(name="p", bufs=5))
        lm_pool = ctx.enter_context(tc.tile_pool(name="lm", bufs=3))
        eq_pool = ctx.enter_context(tc.tile_pool(name="eq", bufs=2))
        msk_pool = ctx.enter_context(tc.tile_pool(name="msk", bufs=2))
        a_pool = ctx.enter_context(tc.tile_pool(name="apool", bufs=2))
        tail_pool = ctx.enter_context(tc.tile_pool(name="tail", bufs=2))
        psum_pool = ctx.enter_context(tc.tile_pool(name="ps", bufs=1, space="PSUM"))

        idn_sb = const.tile([P, P], DT.bfloat16, tag="idn")
        nc.sync.dma_start(out=idn_sb[:], in_=idn[:])

        stats = const.tile([P, NCOLS], DT.float32, tag="stats")
        nc.vector.memset(stats[:], 0.0)

        # bias=-1.0 has no pre-registered const AP; build one
        neg1 = const.tile([P, 1], DT.float32, tag="neg1")
        nc.vector.memset(neg1[:], -1.0)

        t_i32 = const.tile([P, FCOLS], DT.int32, tag="ti")
        nc.sync.dma_start(out=t_i32[:], in_=target[:])
        t_bf = const.tile([P, FCOLS], DT.bfloat16, tag="tb")
        nc.vector.tensor_copy(out=t_bf[:], in_=t_i32[:])

        lsel_ps = psum_pool.tile([P, FCOLS], DT.float32, tag="lsel")

        a_prev = None
        eq_next = None

        def emit_eq(cls, cols_slice, tag="eq"):
            cols = cols_slice.stop - cols_slice.start
            eq = eq_pool.tile([P, cols], DT.bfloat16, tag=tag)
            nc.vector.tensor_scalar(
                out=eq[:],
                in0=t_bf[:, cols_slice],
                scalar1=float(cls),
                scalar2=None,
                op0=mybir.AluOpType.is_equal,
            )
            return eq

        # eq for class 0 up front
        eq_next = emit_eq(0, slice(0, FCOLS))

        # classes 0..17 full-grid
        for c in range(C - 1):
            p_t = p_pool.tile([P, FCOLS], DT.float32, tag="p")
            nc.sync.dma_start(out=p_t[:], in_=pred_r[c, :, :])
            lm = lm_pool.tile([P, FCOLS], DT.bfloat16, tag="lm")
            nc.scalar.activation(
                out=lm[:],
                in_=p_t[:],
                func=mybir.ActivationFunctionType.Ln,
                bias=1.0,
                scale=-1.0,
                accum_out=stats[:, COL_U + c : COL_U + c + 1],
            )
            eq = eq_next
            eq_next = emit_eq(c + 1, slice(0, FCOLS))
            msk = msk_pool.tile([P, FCOLS], DT.bfloat16, tag="msk")
            nc.vector.tensor_mul(out=msk[:], in0=eq[:], in1=lm[:])
            if c == 0:
                a_new = a_pool.tile([P, FCOLS], DT.bfloat16, tag="a")
                nc.vector.tensor_single_scalar(
                    out=a_new[:], in_=lm[:], scalar=1.0, op=mybir.AluOpType.mult
                )
            else:
                a_new = a_pool.tile([P, FCOLS], DT.bfloat16, tag="a")
                nc.vector.tensor_add(out=a_new[:], in0=a_prev[:], in1=lm[:])
            a_prev = a_new
            for s in range(FCOLS // 512):
                ssl = slice(s * 512, (s + 1) * 512)
                nc.tensor.matmul(
                    lsel_ps[:, ssl],
                    lhsT=idn_sb[:],
                    rhs=msk[:, ssl],
                    start=(c == 0),
                    stop=False,
                )

        # class 18 quartered; final A add per quarter; pos*A on GpSimd
        a_sb = const.tile([P, FCOLS], DT.bfloat16, tag="asbF")
        for q in range(4):
            qsl = slice(q * QT, (q + 1) * QT)
            p_t = p_pool.tile([P, QT], DT.float32, tag="p")
            nc.sync.dma_start(out=p_t[:], in_=pred_r[C - 1, :, qsl])
            lm = lm_pool.tile([P, QT], DT.bfloat16, tag="lm")
            nc.scalar.activation(
                out=lm[:],
                in_=p_t[:],
                func=mybir.ActivationFunctionType.Ln,
                bias=1.0,
                scale=-1.0,
                accum_out=stats[:, COL_U + 18 + q : COL_U + 19 + q],
            )
            eq18 = eq_next if q == 0 else emit_eq(C - 1, qsl)
            if q == 0:
                # eq_next was emitted full-grid for class 18; slice it
                eq18 = eq_next
                eqv = eq18[:, qsl]
            else:
                eqv = eq18[:]
            msk = msk_pool.tile([P, QT], DT.bfloat16, tag="msk")
            nc.vector.tensor_mul(out=msk[:], in0=eqv, in1=lm[:])
            nc.vector.tensor_add(out=a_sb[:, qsl], in0=a_prev[:, qsl], in1=lm[:])
            for s in range(2):
                ssl = slice(q * QT + s * 512, q * QT + (s + 1) * 512)
                nc.tensor.matmul(
                    lsel_ps[:, ssl],
                    lhsT=idn_sb[:],
                    rhs=msk[:, s * 512 : (s + 1) * 512],
                    start=False,
                    stop=True,
                )
            # ---- tail for this quarter ----
            scr = tail_pool.tile([P, QT], DT.bfloat16, tag="scr")
            nc.vector.scalar_tensor_tensor(
                out=scr[:],
                in0=t_bf[:, qsl],
                scalar=0.5,
                in1=a_sb[:, qsl],
                op0=mybir.AluOpType.is_gt,
                op1=mybir.AluOpType.mult,
                accum_out=stats[:, COL_POSA + q : COL_POSA + q + 1],
            )
            expn = tail_pool.tile([P, QT], DT.float32, tag="expn")
            nc.scalar.activation(
                out=expn[:],
                in_=lsel_ps[:, qsl],
                func=mybir.ActivationFunctionType.Exp,
                scale=-1.0,
            )
            b_t = tail_pool.tile([P, QT], DT.bfloat16, tag="b")
            nc.scalar.activation(
                out=b_t[:],
                in_=expn[:],
                func=mybir.ActivationFunctionType.Ln,
                bias=neg1[:],
                accum_out=stats[:, COL_SUMB + q : COL_SUMB + q + 1],
            )
            scrb = tail_pool.tile([P, QT], DT.bfloat16, tag="scrb")
            nc.vector.scalar_tensor_tensor(
                out=scrb[:],
                in0=t_bf[:, qsl],
                scalar=0.5,
                in1=b_t[:],
                op0=mybir.AluOpType.is_gt,
                op1=mybir.AluOpType.mult,
                accum_out=stats[:, COL_POSB + q : COL_POSB + q + 1],
            )

        nc.sync.dma_start(out=out[:], in_=stats[:])

    if not nc.is_finalized():
        nc.finalize()

    return nc


def combine(outs, pos_count, n_cores) -> np.float32:
    """Fold the cores' [128, 40] stats tiles into the scalar loss.
    pos_count is computed on host from the target input."""
    tot = np.float64(PIX) * n_cores
    pos = np.float64(pos_count)
    s_all = np.float64(0.0)
    s_pos = np.float64(0.0)
    for st in outs:
        st = st.astype(np.float64)
        u_all = st[:, COL_U : COL_U + 22].sum()
        pos_a = st[:, COL_POSA : COL_POSA + 4].sum()
        pos_b = st[:, COL_POSB : COL_POSB + 4].sum()
        sum_b = st[:, COL_SUMB : COL_SUMB + 4].sum()
        s_all += -(sum_b + u_all)
        s_pos += -(pos_b + pos_a)
    neg = tot - pos
    s_neg = s_all - s_pos
    loss = ((neg / tot) * s_pos + (pos / tot) * s_neg) / (tot * C)
    return np.float32(loss)


_NC_CACHE = None


def kernel(predict: np.ndarray, target: np.ndarray) -> np.ndarray:
    global _NC_CACHE
    if _NC_CACHE is None:
        _NC_CACHE = build_kernel()
    nc = _NC_CACHE

    import ml_dtypes

    predict = np.ascontiguousarray(predict, dtype=np.float32)
    target = np.ascontiguousarray(target, dtype=np.int32)
    idn = np.eye(P, dtype=np.float32).astype(ml_dtypes.bfloat16)

    in_maps = []
    for k in range(N_CORES):
        in_maps.append(
            {
                "predict": predict[k].reshape(C, PIX),
                "target": target[k].reshape(P, FCOLS),
                "idn": idn,
            }
        )

    res = run_bass_kernel_spmd(nc, in_maps, list(range(N_CORES)))
    pos_count = int(np.count_nonzero(target))
    return combine(
        [res.results[k]["out"] for k in range(N_CORES)], pos_count, N_CORES
    )


# revision 20
# speedup vs baseline: 1.1466x; 1.1133x over previous
"""Weighted 2D cross-entropy (BCE-over-classes) loss on 8 Trainium2 cores.

Math (matches the reference):
  t in [0,19); pos = t>0, neg = t==0 (all pixels are pos or neg; mask == 1)
  S(i) = sum_c bce(i,c) = -[ B(i) + A(i) ]
     A(i) = sum_c log(1-p_c(i))
     B(i) = log(p_t(i)) - log(1-p_t(i)) = Ln(exp(-L_sel(i)) - 1)
  loss = ( (NEG/TOT)*S_pos_sum + (POS/TOT)*S_neg_sum ) / (TOT*C)

Per-core (core k <- batch element k, pure data parallel), baseline
quarter-tile pipeline with three surgical deltas to rebalance engines:
  - one full-grid ACT pass: L_c = Ln(1-p_c) in bf16
  - one full-grid DVE pass: eq at 4x + masked mult at 2x
  - PE identity-matmuls accumulate A and L_sel into PSUM f32, EXCEPT
    classes 0..SPLIT-1 whose A-contribution is summed on DVE in bf16
    (ping-pong adds) and injected into the PSUM group by one extra
    matmul pair per tile -- balances PE (~-17us) against DVE (~+13us)
  - tail per tile: B = Ln(exp(-L_sel)-1) directly (two chained ACTs,
    no logp tile, fewer DVE reductions); pos-masked sums via STT
  - pos count and the 128-partition fold happen on host (target is a
    kernel input; stats go out as [128, 16] f32)
"""

from contextlib import ExitStack

import numpy as np

import concourse.bass as bass
import concourse.mybir as mybir
import concourse.tile as tile
from concourse import bacc
from concourse.bass_utils import run_bass_kernel_spmd

# problem shape (hardcoded per harness contract)
N, C, H, W = 8, 19, 512, 1024
PIX = H * W          # 524288 pixels per core
P = 128              # partitions
FCOLS = PIX // P     # 4096 free columns when pixels laid out [128, 4096]
FT = 1024            # pixel-tile free width
NTILES = FCOLS // FT # 4 pixel tiles per core
N_CORES = 8
SPLIT = 5            # classes 0..4: A accumulated on DVE instead of PE

DT = mybir.dt

# stats column layout (all f32; one column per (stat, tile))
COL_UALL = 0         # sum A per tile (PSUM A incl. injected DVE partial)
COL_POSA = 4         # sum pos*A per tile
COL_SUMB = 8         # sum B per tile
COL_POSB = 12        # sum pos*B per tile
NCOLS = 16


def build_kernel() -> bass.Bass:
    # Bacc (not raw Bass): its compile() pipeline runs
    # generate_event_semaphores, which splits multi-sem waits to satisfy the
    # 1-wait-per-instruction TRN2 sync structs -- raw Bass modules with
    # Tile-emitted multi-waits fail walrus codegen.
    nc = bacc.Bacc("TRN2")

    predict = nc.declare_dram_parameter("predict", [C, PIX], DT.float32, isOutput=False)
    target = nc.declare_dram_parameter("target", [P, FCOLS], DT.int32, isOutput=False)
    idn = nc.declare_dram_parameter("idn", [P, P], DT.bfloat16, isOutput=False)
    out = nc.declare_dram_parameter("out", [P, NCOLS], DT.float32, isOutput=True)

    pred_r = predict.rearrange("c (p f) -> c p f", p=P)  # [19, 128, 4096]

    with tile.TileContext(nc) as tc, ExitStack() as ctx:
        const = ctx.enter_context(tc.tile_pool(name="const", bufs=1))
        p_pool = ctx.enter_context(tc.tile_pool(name="p", bufs=8))
        lm_pool = ctx.enter_context(tc.tile_pool(name="lm", bufs=21))
        pix_pool = ctx.enter_context(tc.tile_pool(name="pix", bufs=2))
        scr_pool = ctx.enter_context(tc.tile_pool(name="scr", bufs=2))
        eq_pool = ctx.enter_context(tc.tile_pool(name="eq", bufs=4))
        a_pool = ctx.enter_context(tc.tile_pool(name="adve", bufs=2))
        psum_pool = ctx.enter_context(tc.tile_pool(name="ps", bufs=2, space="PSUM"))

        idn_sb = const.tile([P, P], DT.bfloat16, tag="idn")
        nc.sync.dma_start(out=idn_sb[:], in_=idn[:])

        t_i32 = const.tile([P, FCOLS], DT.int32, tag="ti")
        nc.sync.dma_start(out=t_i32[:], in_=target[:])
        t_bf = const.tile([P, FCOLS], DT.bfloat16, tag="tb")
        nc.vector.tensor_copy(out=t_bf[:], in_=t_i32[:])

        stats = const.tile([P, NCOLS], DT.float32, tag="stats")
        nc.vector.memset(stats[:], 0.0)

        # bias=-1.0 has no pre-registered const AP; build one
        neg1 = const.tile([P, 1], DT.float32, tag="neg1")
        nc.vector.memset(neg1[:], -1.0)

        for t in range(NTILES):
            fsl = slice(t * FT, (t + 1) * FT)
            t_sl = t_bf[:, fsl]

            # PSUM accumulator: [:, :FT] = A, [:, FT:] = L_sel   (4 banks)
            acc_ps = psum_pool.tile([P, 2 * FT], DT.float32, tag="acc")

            a_dve = None
            for c in range(C):
                p_t = p_pool.tile([P, FT], DT.float32, tag="p")
                # p bufs=8 aligns slot reuse with the global DMA->DMAHW-proc
                # round-robin (8 procs), so the WAW on the old writer is
                # same-proc FIFO order and Tile emits no cross-queue wait
                nc.sync.dma_start(out=p_t[:], in_=pred_r[c, :, fsl])

                # lm[:, :FT] = L_c = Ln(1-p) bf16 ; lm[:, FT:] = (T==c)*L_c
                lm = lm_pool.tile([P, 2 * FT], DT.bfloat16, tag="lm")
                nc.scalar.activation(
                    out=lm[:, :FT],
                    in_=p_t[:],
                    func=mybir.ActivationFunctionType.Ln,
                    bias=1.0,
                    scale=-1.0,
                )
                # eq at DVE 4x (16-bit tensor_scalar) + mult at 2x beats the
                # fused scalar_tensor_tensor, which only has a 1x uop
                eq = eq_pool.tile([P, FT], DT.bfloat16, tag="eq")
                nc.vector.tensor_scalar(
                    out=eq[:],
                    in0=t_sl,
                    scalar1=float(c),
                    scalar2=None,
                    op0=mybir.AluOpType.is_equal,
                )
                nc.vector.tensor_mul(out=lm[:, FT:], in0=eq[:], in1=lm[:, :FT])

                if c < SPLIT:
                    # A for the first SPLIT classes on DVE (bf16 ping-pong)
                    if c == 0:
                        a_new = a_pool.tile([P, FT], DT.bfloat16, tag="a")
                        nc.vector.tensor_single_scalar(
                            out=a_new[:],
                            in_=lm[:, :FT],
                            scalar=1.0,
                            op=mybir.AluOpType.mult,
                        )
                    else:
                        a_new = a_pool.tile([P, FT], DT.bfloat16, tag="a")
                        nc.vector.tensor_add(
                            out=a_new[:], in0=a_dve[:], in1=lm[:, :FT]
                        )
                    a_dve = a_new
                    # only the select-half goes to PE for these classes
                    for s in (2, 3):
                        ssl = slice(s * 512, (s + 1) * 512)
                        nc.tensor.matmul(
                            acc_ps[:, ssl],
                            lhsT=idn_sb[:],
                            rhs=lm[:, ssl],
                            start=(c == 0),
                            stop=(c == C - 1),
                        )
                else:
                    for s in range(4):
                        ssl = slice(s * 512, (s + 1) * 512)
                        nc.tensor.matmul(
                            acc_ps[:, ssl],
                            lhsT=idn_sb[:],
                            rhs=lm[:, ssl],
                            start=(c == SPLIT if s < 2 else False),
                            stop=(c == C - 1 if s >= 2 else False),
                        )

            # inject the DVE A-partial into the PSUM A-half (closes group)
            for s in (0, 1):
                ssl = slice(s * 512, (s + 1) * 512)
                nc.tensor.matmul(
                    acc_ps[:, ssl],
                    lhsT=idn_sb[:],
                    rhs=a_dve[:, ssl],
                    start=False,
                    stop=True,
                )

            a_ps = acc_ps[:, :FT]
            lsel_ps = acc_ps[:, FT:]

            # ---- tail ----
            # sum A
            nc.vector.tensor_reduce(
                out=stats[:, COL_UALL + t : COL_UALL + t + 1],
                in_=a_ps,
                axis=mybir.AxisListType.X,
                op=mybir.AluOpType.add,
            )
            # sum pos*A
            scr = scr_pool.tile([P, FT], DT.float32, tag="scr")
            nc.vector.scalar_tensor_tensor(
                out=scr[:],
                in0=t_sl,
                scalar=0.5,
                in1=a_ps,
                op0=mybir.AluOpType.is_gt,
                op1=mybir.AluOpType.mult,
                accum_out=stats[:, COL_POSA + t : COL_POSA + t + 1],
            )
            # B = Ln(exp(-L_sel) - 1): expn = 1/(1-p_t), then Ln(expn-1)
            expn = pix_pool.tile([P, FT], DT.float32, tag="expn")
            nc.scalar.activation(
                out=expn[:],
                in_=lsel_ps,
                func=mybir.ActivationFunctionType.Exp,
                scale=-1.0,
            )
            b_t = pix_pool.tile([P, FT], DT.bfloat16, tag="b")
            nc.scalar.activation(
                out=b_t[:],
                in_=expn[:],
                func=mybir.ActivationFunctionType.Ln,
                bias=neg1[:],
            )
            # sum B
            nc.vector.tensor_reduce(
                out=stats[:, COL_SUMB + t : COL_SUMB + t + 1],
                in_=b_t[:],
                axis=mybir.AxisListType.X,
                op=mybir.AluOpType.add,
            )
            # sum pos*B
            scrb = scr_pool.tile([P, FT], DT.bfloat16, tag="scrb")
            nc.vector.scalar_tensor_tensor(
                out=scrb[:],
                in0=t_sl,
                scalar=0.5,
                in1=b_t[:],
                op0=mybir.AluOpType.is_gt,
                op1=mybir.AluOpType.mult,
                accum_out=stats[:, COL_POSB + t : COL_POSB + t + 1],
            )

        nc.sync.dma_start(out=out[:], in_=stats[:])

    if not nc.is_finalized():
        nc.finalize()

    return nc


def combine(outs, pos_count, n_cores) -> np.float32:
    """Fold the cores' [128, 16] stats tiles into the scalar loss.
    pos_count comes from the target input on host."""
    tot = np.float64(PIX) * n_cores
    pos = np.float64(pos_count)
    s_all = np.float64(0.0)
    s_pos = np.float64(0.0)
    for st in outs:
        st = st.astype(np.float64)
        u_all = st[:, COL_UALL : COL_UALL + NTILES].sum()
        pos_a = st[:, COL_POSA : COL_POSA + NTILES].sum()
        sum_b = st[:, COL_SUMB : COL_SUMB + NTILES].sum()
        pos_b = st[:, COL_POSB : COL_POSB + NTILES].sum()
        s_all += -(sum_b + u_all)
        s_pos += -(pos_b + pos_a)
    neg = tot - pos
    s_neg = s_all - s_pos
    loss = ((neg / tot) * s_pos + (pos / tot) * s_neg) / (tot * C)
    return np.float32(loss)


_NC_CACHE = None


def kernel(predict: np.ndarray, target: np.ndarray) -> np.ndarray:
    global _NC_CACHE
    if _NC_CACHE is None:
        _NC_CACHE = build_kernel()
    nc = _NC_CACHE

    import ml_dtypes

    predict = np.ascontiguousarray(predict, dtype=np.float32)
    target = np.ascontiguousarray(target, dtype=np.int32)
    idn = np.eye(P, dtype=np.float32).astype(ml_dtypes.bfloat16)

    in_maps = []
    for k in range(N_CORES):
        in_maps.append(
            {
                "predict": predict[k].reshape(C, PIX),
                "target": target[k].reshape(P, FCOLS),
                "idn": idn,
            }
        )

    res = run_bass_kernel_spmd(nc, in_maps, list(range(N_CORES)))
    pos_count = int(np.count_nonzero(target))
    return combine(
        [res.results[k]["out"] for k in range(N_CORES)], pos_count, N_CORES
    )
